# revision 39
# baseline (speedup 1.0000x reference)
"""Causal self-attention with RoPE on 8 Trainium2 NeuronCores.

Problem: B=4, T=2048, C=1024, 16 heads x 64 dim, fp32 reference.

Sharding: 8 cores = (batch b in 0..3) x (head-group g in 0..1, 8 heads each).
Each core computes qkv for its batch/head-slice (column-parallel qkv),
full attention for its 8 heads, and a row-parallel partial projection.
Host sums the two partial projections per batch (the "all-reduce").

Per-core kernel layout strategy (v2 — software-pipelined):
  - q/k are produced DIRECTLY in transposed [d, t] layout by running the
    qkv matmul in [f, t] orientation (lhsT = W slab, rhs = xT slab), so
    no PE transposes are needed at all.
  - RoPE is applied in the transposed layout: rot_half becomes a
    32-partition block swap, done with two DVE multiplies against
    host-precomputed [d, t] cos/sin tables (sign folded into the sin
    table), an SBUF->SBUF DMA partition swap, and one DVE add.
  - v runs in the baseline [t, f] orientation straight into V1 (with a
    ones column appended so attn@v row 64 accumulates the softmax
    denominator for free).
  - Scores are computed TRANSPOSED: ST[tk, tq] = kT.T @ qT per head;
    the two heads of a pair run concurrently in the PE array via row
    tiling (tile_position), writing the two halves of ONE 2-bank PSUM
    tile [128, 1024].
  - exp runs on ScalarE as a single [128, 1024] ACTIVATE per (pair, m)
    straight out of PSUM (scale=1/8 folded in; no max subtraction:
    |scores|/8 << 88, safe in fp32/bf16 range).
  - Causal masking: ONE DVE multiply with a precomputed mask tile per
    diagonal-straddling [128, 1024] exp tile (both heads at once).
  - Normalization: softmax denominator row 64 -> fast reciprocal on
    DVE, gpsimd partition_broadcast, DVE multiply into ONORM.
  - proj: row-parallel y_partial = ONORM.T @ wprojT, fp32 output.
  - EMISSION IS SOFTWARE-PIPELINED: attention block j is interleaved
    with the qkv chains of block j+1 and the projection of block j-1,
    so the PE never starves (keeps the HAM clock gate at 2.4 GHz) and
    ScalarE exp overlaps PE matmul work throughout.
"""

import sys
import threading

sys.path.insert(0, "/opt/trn_rl_repo")

import ml_dtypes
import numpy as np

import concourse.bass as bass
import concourse.mybir as mybir
from concourse import bacc
from concourse.bass_utils import run_bass_kernel_spmd
from concourse.tile import TileContext

BF16 = ml_dtypes.bfloat16
F32 = mybir.dt.float32
BF = mybir.dt.bfloat16

B, T, C = 4, 2048, 1024
NH, D = 16, 64          # global heads
HL = 8                  # local heads per core
G = 2                   # head groups (cores per batch)
FL = 3 * HL * D         # 1536 local qkv rows
CL = HL * D             # 512 local out channels
P = 128
TQ = 512                # query-block width
NTT = T // P            # 16 t-tiles
NPAIR = HL // 2         # 4 head pairs
NB = T // TQ            # 4 query blocks
KO = C // P             # 8 contraction slabs


def build_nc():
    nc = bacc.Bacc("TRN2", target_bir_lowering=False, debug=False, num_devices=8)

    xT = nc.declare_dram_parameter("xT", [C, T], BF, isOutput=False)
    wqkvT = nc.declare_dram_parameter("wqkvT", [C, FL], BF, isOutput=False)
    wprojT = nc.declare_dram_parameter("wprojT", [CL, C], BF, isOutput=False)
    cosd = nc.declare_dram_parameter("cosd", [P, T], F32, isOutput=False)
    sinu = nc.declare_dram_parameter("sinu", [P, T], F32, isOutput=False)
    y = nc.declare_dram_parameter("y", [T, C], F32, isOutput=True)

    Exp = mybir.ActivationFunctionType.Exp

    with TileContext(nc) as tc:
        with (
            tc.tile_pool(name="const", bufs=1) as const,
            tc.tile_pool(name="rope", bufs=4) as rope,
            tc.tile_pool(name="pexp", bufs=8) as pexp,
            tc.tile_pool(name="yout", bufs=3) as yout,
            tc.tile_pool(name="nrm", bufs=2) as nrm,
            tc.tile_pool(name="psscore", bufs=2, space="PSUM") as psscore,
            tc.tile_pool(name="psout", bufs=1, space="PSUM") as psout,
            tc.tile_pool(name="psmm", bufs=2, space="PSUM") as psmm,
        ):
            # ---- persistent SBUF tensors ----
            XT = const.tile([P, KO, T], BF, tag="XT")
            WQKV = const.tile([P, KO, FL], BF, tag="WQKV")
            WPROJ = const.tile([P, CL // P, C], BF, tag="WPROJ")
            COS = const.tile([P, T], F32, tag="COS")
            SINU = const.tile([P, T], F32, tag="SINU")
            V1 = const.tile([P, NTT, HL, D + 1], BF, tag="V1")
            QT = const.tile([P, NPAIR, T], BF, tag="QT")
            KT = const.tile([P, NPAIR, T], BF, tag="KT")
            ONORM = const.tile([P, NPAIR, T], BF, tag="ONORM")
            MTRI = const.tile([P, 2, TQ], BF, tag="MTRI")
            WARM = const.tile([P, TQ], BF, tag="WARM")

            # ---- input DMAs: first-needed data issues first (the 16 HW
            # queues run transfers concurrently, so late loads are only
            # delayed, never starved) ----
            xTr = xT.rearrange("(ko p) t -> p ko t", p=P)
            wqr = wqkvT.rearrange("(ko p) f -> p ko f", p=P)
            # Only t-block-0 data + the rope tables load up front; later
            # t-blocks and wproj are spliced into the pipeline as fillers
            # so the sync queue never head-of-line-blocks the rope shift
            # DMAs of the active block.
            # Weights load as f-slices in first-use order: the first qk
            # chain only needs its own 256KB column slice, so compute
            # starts ~3x earlier than with whole-slab loads.
            def emit_wq_load(fb, w):
                nc.sync.dma_start(WQKV[:, :, fb:fb + w], wqr[:, :, fb:fb + w])

            emit_wq_load(0, P)                # q, pair 0
            for ko in range(KO):
                nc.sync.dma_start(XT[:, ko, 0:TQ], xTr[:, ko, 0:TQ])
            emit_wq_load(CL, P)               # k, pair 0
            emit_wq_load(2 * CL, CL)          # v, all heads
            nc.sync.dma_start(COS[:], cosd[:, :])
            nc.sync.dma_start(SINU[:], sinu[:, :])
            for p4 in range(1, NPAIR):
                emit_wq_load(p4 * P, P)
                emit_wq_load(CL + p4 * P, P)

            def emit_xt_load(tb):
                for ko in range(KO):
                    nc.sync.dma_start(
                        XT[:, ko, tb * TQ:(tb + 1) * TQ],
                        xTr[:, ko, tb * TQ:(tb + 1) * TQ])

            def emit_wproj_load():
                nc.sync.dma_start(
                    WPROJ[:], wprojT.rearrange("(ko p) o -> p ko o", p=P))

            # Warm-up: dense dependency-free matmuls on a memset tile keep
            # the PE's HAM clock gate at 8/8 while the input DMAs land
            # (results land in a scratch psum slot and are overwritten).
            nc.gpsimd.memset(WARM[:], 1.0)
            wps = psmm.tile([P, TQ], F32, tag="mm")
            for _ in range(16):
                nc.tensor.matmul(wps[:], lhsT=WARM[:, 0:P], rhs=WARM[:],
                                 start=True, stop=True)

            nc.gpsimd.memset(V1[:], 1.0)
            nc.gpsimd.memset(MTRI[:], 1.0)
            for h in range(2):  # universal triangular mask: keep f >= p
                nc.gpsimd.affine_select(
                    out=MTRI[:, h, :], in_=MTRI[:, h, :],
                    compare_op=mybir.AluOpType.is_ge, fill=0.0,
                    base=0, pattern=[[1, TQ]], channel_multiplier=-1)

            yr = y.rearrange("(n p) o -> p n o", p=P)

            # ---- emission helpers ----
            def emit_qk_chain(j, p4, qk):
                """Produce QT/KT pair p4 for t-block j, already transposed,
                with RoPE fused into the PSUM evacuation."""
                tsl = slice(j * TQ, (j + 1) * TQ)
                fb = qk * CL + p4 * P
                ps = psmm.tile([P, TQ], F32, tag="mm")
                for ko in range(KO):
                    nc.tensor.matmul(
                        ps[:], lhsT=WQKV[:, ko, fb:fb + P],
                        rhs=XT[:, ko, tsl],
                        start=(ko == 0), stop=(ko == KO - 1))
                a = rope.tile([P, TQ], BF, tag="ra")
                b = rope.tile([P, TQ], BF, tag="rb")
                bs = rope.tile([P, TQ], BF, tag="rs")
                nc.vector.tensor_mul(a[:], ps[:], COS[:, tsl])
                nc.vector.tensor_mul(b[:], ps[:], SINU[:, tsl])
                # rot_half = swap 32-partition blocks within each head
                for blk in range(4):
                    src = (blk ^ 1) * 32
                    nc.sync.dma_start(
                        bs[blk * 32:(blk + 1) * 32, :], b[src:src + 32, :])
                dst = QT if qk == 0 else KT
                nc.vector.tensor_add(dst[:, p4, tsl], a[:], bs[:])

            def emit_v_chain(i):
                ps = psmm.tile([P, TQ], F32, tag="mm")
                for ko in range(KO):
                    nc.tensor.matmul(
                        ps[:], lhsT=XT[:, ko, i * P:(i + 1) * P],
                        rhs=WQKV[:, ko, 2 * CL:3 * CL],
                        start=(ko == 0), stop=(ko == KO - 1))
                nc.vector.tensor_copy(
                    V1[:, i, :, 0:D], ps.rearrange("p (h d) -> p h d", h=HL))

            def emit_att_m(j, p4, m, ntk, outAB):
                # Diagonal-straddling tiles (ml >= 0) only contribute to
                # queries tq >= 128*ml within the block: shrink the score
                # matmuls, exp, mask, and attn@v to that span.  Since the
                # span offset equals 128*ml, the causal mask reduces to a
                # universal triangle (keep f >= p) for every such tile.
                ml = m - (TQ // P) * j
                qoff = max(ml, 0) * P
                nq = TQ - qoff
                qsl = slice(j * TQ + qoff, (j + 1) * TQ)
                ksl = slice(m * P, (m + 1) * P)
                st = psscore.tile([P, 2 * TQ], F32, tag="st")
                st2 = st.rearrange("p (h q) -> p h q", h=2)
                nc.tensor.matmul(
                    st2[:, 0, qoff:TQ], lhsT=KT[0:D, p4, ksl],
                    rhs=QT[0:D, p4, qsl],
                    start=True, stop=True)
                nc.tensor.matmul(
                    st2[:, 1, qoff:TQ], lhsT=KT[D:P, p4, ksl],
                    rhs=QT[D:P, p4, qsl],
                    start=True, stop=True, tile_position=(D, 0))
                pt = pexp.tile([P, 2 * TQ], BF, tag="pt")
                pt2 = pt.rearrange("p (h q) -> p h q", h=2)
                nc.scalar.activation(
                    pt2[:, :, qoff:TQ], st2[:, :, qoff:TQ], Exp, scale=0.125)
                if ml >= 0:  # zero tk > tq on the diagonal tile
                    nc.vector.tensor_mul(
                        pt2[:, :, qoff:TQ], pt2[:, :, qoff:TQ],
                        MTRI[:, :, 0:nq])
                nc.tensor.matmul(
                    outAB[0:D + 1, qoff:TQ], lhsT=V1[:, m, 2 * p4, :],
                    rhs=pt2[:, 0, qoff:TQ],
                    start=(m == 0), stop=(m == ntk - 1))
                nc.tensor.matmul(
                    outAB[0:D + 1, TQ + qoff:2 * TQ],
                    lhsT=V1[:, m, 2 * p4 + 1, :],
                    rhs=pt2[:, 1, qoff:TQ],
                    start=(m == 0), stop=(m == ntk - 1))

            def emit_norm(j, p4, outAB):
                qsl = slice(j * TQ, (j + 1) * TQ)
                l2 = nrm.tile([1, 2 * TQ], F32, tag="l2")
                nc.vector.tensor_copy(l2[:], outAB[D:D + 1, :])
                r2 = nrm.tile([1, 2 * TQ], F32, tag="r2")
                nc.vector.reciprocal_approx_fast(out=r2[:], in_=l2[:])
                for w in range(2):
                    r64 = nrm.tile([D, TQ], F32, tag="r64")
                    nc.gpsimd.partition_broadcast(
                        r64[:], r2[:, w * TQ:(w + 1) * TQ])
                    nc.vector.tensor_mul(
                        ONORM[w * D:(w + 1) * D, p4, qsl],
                        outAB[0:D, w * TQ:(w + 1) * TQ], r64[:])

            def emit_proj(i, n2):
                ps = psmm.tile([P, TQ], F32, tag="mm")
                for kc in range(NPAIR):
                    nc.tensor.matmul(
                        ps[:], lhsT=ONORM[:, kc, i * P:(i + 1) * P],
                        rhs=WPROJ[:, kc, n2 * TQ:(n2 + 1) * TQ],
                        start=(kc == 0), stop=(kc == NPAIR - 1))
                ysb = yout.tile([P, TQ], F32, tag="ysb")
                nc.vector.tensor_copy(ysb[:], ps[:])
                # output stores ride the software DGE: latency-tolerant,
                # keeps the sync queue free for rope shifts
                nc.gpsimd.dma_start(yr[:, i, n2 * TQ:(n2 + 1) * TQ], ysb[:])

            def q_fillers(j):
                return [lambda j=j, p4=p4: emit_qk_chain(j, p4, 0)
                        for p4 in range(NPAIR)]

            def qk_fillers(j):
                # pair-interleaved so pair 0 of the next block completes
                # first (its scores unlock that block's exp stream)
                f = []
                for p4 in range(NPAIR):
                    f.append(lambda j=j, p4=p4: emit_qk_chain(j, p4, 0))
                    f.append(lambda j=j, p4=p4: emit_qk_chain(j, p4, 1))
                return f

            def kv_fillers(j):
                f = [lambda j=j, p4=p4: emit_qk_chain(j, p4, 1)
                     for p4 in range(NPAIR)]
                f += [lambda i=j * (TQ // P) + s: emit_v_chain(i)
                      for s in range(TQ // P)]
                return f

            def v_fillers(j):
                return [lambda i=j * (TQ // P) + s: emit_v_chain(i)
                        for s in range(TQ // P)]

            def proj_block_fillers(j):
                return [lambda i=i, n2=n2: emit_proj(i, n2)
                        for i in range(j * (TQ // P), (j + 1) * (TQ // P))
                        for n2 in range(C // TQ)]

            # ---- software-pipelined main loop ----
            # Fillers (independent PE chains) are spliced between the
            # ScalarE-bound attention iterations of each block so the PE
            # never starves.  Later blocks are ACT-bound, so the late
            # qkv/proj chains are pulled as early as dependencies allow.

            # attention(0) starts as soon as pair 0 + v of block 0 exist;
            # the q/k chains of later pairs are spliced into earlier groups
            emit_qk_chain(0, 0, 0)
            emit_qk_chain(0, 0, 1)
            for s in range(TQ // P):
                emit_v_chain(s)
            emit_xt_load(1)
            filler_map = {
                0: [lambda: emit_xt_load(2)] + qk_fillers(1) + v_fillers(1),
                1: [lambda: emit_xt_load(3), emit_wproj_load]
                   + qk_fillers(2) + v_fillers(2) + q_fillers(3),
                2: kv_fillers(3) + proj_block_fillers(0),
                3: proj_block_fillers(1) + proj_block_fillers(2),
            }
            carry = []
            for j in range(NB):
                fillers = carry + filler_map[j]
                ntk = (TQ // P) * (j + 1)
                # front-load fillers over ~70% of the block so qkv(j+1)
                # finishes before attention(j) drains ScalarE's queue
                natt = (NPAIR * ntk * 7) // 10
                fi = 0
                k = 0
                for p4 in range(NPAIR):
                    outAB = psout.tile([D + 1, 2 * TQ], F32, tag="out")
                    for m in range(ntk):
                        emit_att_m(j, p4, m, ntk, outAB)
                        if j == 0 and p4 < NPAIR - 1 and m < 2:
                            emit_qk_chain(0, p4 + 1, m)
                        k += 1
                        while fi < len(fillers) and fi * natt < k * len(fillers):
                            fillers[fi]()
                            fi += 1
                    emit_norm(j, p4, outAB)
                # leftovers ride along with the next block's fillers so the
                # next block's score/exp stream is not delayed behind them
                carry = fillers[fi:]
            for fn in carry:
                fn()
            for fn in proj_block_fillers(NB - 1):
                fn()

    nc.compile()
    return nc


def prep_inputs(x, w_qkv, w_proj):
    """Build the 8 per-core input maps from the full-problem inputs."""
    x = np.asarray(x, dtype=np.float32)
    w_qkv = np.asarray(w_qkv, dtype=np.float32)
    w_proj = np.asarray(w_proj, dtype=np.float32)

    inv_freq = 1.0 / (10000.0 ** (np.arange(0, D, 2, dtype=np.float32) / D))
    tt = np.arange(T, dtype=np.float32)
    freqs = np.outer(tt, inv_freq).astype(np.float32)  # [T, 32]
    cos_t = np.cos(freqs).astype(np.float32)           # [T, 32]
    sin_t = np.sin(freqs).astype(np.float32)
    # [d, t] tables for the transposed layout, stacked for a head pair.
    # cosd[p, t] = cos(f[t, p%32]); sinu carries rot_half's sign:
    # +sin for p%64 in [0,32) (source for upper target), -sin for [32,64).
    cos64 = np.concatenate([cos_t.T, cos_t.T], axis=0)   # [64, T]
    sin64 = np.concatenate([sin_t.T, -sin_t.T], axis=0)  # [64, T]
    cosd = np.ascontiguousarray(
        np.concatenate([cos64, cos64], axis=0), dtype=np.float32)  # [128, T]
    sinu = np.ascontiguousarray(
        np.concatenate([sin64, sin64], axis=0), dtype=np.float32)

    in_maps = []
    for core in range(8):
        b, g = divmod(core, G)
        sl = slice(g * CL, (g + 1) * CL)
        w_local = np.concatenate(
            [w_qkv[sl], w_qkv[C:][sl], w_qkv[2 * C:][sl]], axis=0)  # [1536, C]
        in_maps.append({
            "xT": np.ascontiguousarray(x[b].T).astype(BF16),
            "wqkvT": np.ascontiguousarray(w_local.T).astype(BF16),
            "wprojT": np.ascontiguousarray(w_proj[:, sl].T).astype(BF16),
            "cosd": cosd,
            "sinu": sinu,
        })
    return in_maps


_NC_LOCK = threading.Lock()
_NC = None


def get_nc():
    global _NC
    with _NC_LOCK:
        if _NC is None:
            _NC = build_nc()
    return _NC


def run(nc, in_maps, **kw):
    res = run_bass_kernel_spmd(nc, in_maps, list(range(8)), **kw)
    parts = [res.results[c]["y"] for c in range(8)]
    out = np.stack([parts[2 * b] + parts[2 * b + 1] for b in range(B)])
    return out.astype(np.float32), res


def kernel(x, w_qkv, w_proj):
    out, _ = run(get_nc(), prep_inputs(x, w_qkv, w_proj))
    return out


# revision 40
# speedup vs baseline: 1.0004x; 1.0004x over previous
"""Causal self-attention with RoPE on 8 Trainium2 NeuronCores.

Problem: B=4, T=2048, C=1024, 16 heads x 64 dim, fp32 reference.

Sharding: 8 cores = (batch b in 0..3) x (head-group g in 0..1, 8 heads each).
Each core computes qkv for its batch/head-slice (column-parallel qkv),
full attention for its 8 heads, and a row-parallel partial projection.
Host sums the two partial projections per batch (the "all-reduce").

Per-core kernel layout strategy (v2 — software-pipelined):
  - q/k are produced DIRECTLY in transposed [d, t] layout by running the
    qkv matmul in [f, t] orientation (lhsT = W slab, rhs = xT slab), so
    no PE transposes are needed at all.
  - RoPE is applied in the transposed layout: rot_half becomes a
    32-partition block swap, done with two DVE multiplies against
    host-precomputed [d, t] cos/sin tables (sign folded into the sin
    table), an SBUF->SBUF DMA partition swap, and one DVE add.
  - v runs in the baseline [t, f] orientation straight into V1 (with a
    ones column appended so attn@v row 64 accumulates the softmax
    denominator for free).
  - Scores are computed TRANSPOSED: ST[tk, tq] = kT.T @ qT per head;
    the two heads of a pair run concurrently in the PE array via row
    tiling (tile_position), writing the two halves of ONE 2-bank PSUM
    tile [128, 1024].
  - exp runs on ScalarE as a single [128, 1024] ACTIVATE per (pair, m)
    straight out of PSUM (scale=1/8 folded in; no max subtraction:
    |scores|/8 << 88, safe in fp32/bf16 range).
  - Causal masking: ONE DVE multiply with a precomputed mask tile per
    diagonal-straddling [128, 1024] exp tile (both heads at once).
  - Normalization: softmax denominator row 64 -> fast reciprocal on
    DVE, gpsimd partition_broadcast, DVE multiply into ONORM.
  - proj: row-parallel y_partial = ONORM.T @ wprojT, fp32 output.
  - EMISSION IS SOFTWARE-PIPELINED: attention block j is interleaved
    with the qkv chains of block j+1 and the projection of block j-1,
    so the PE never starves (keeps the HAM clock gate at 2.4 GHz) and
    ScalarE exp overlaps PE matmul work throughout.
"""

import sys
import threading

sys.path.insert(0, "/opt/trn_rl_repo")

import ml_dtypes
import numpy as np

import concourse.bass as bass
import concourse.mybir as mybir
from concourse import bacc
from concourse.bass_utils import run_bass_kernel_spmd
from concourse.tile import TileContext

BF16 = ml_dtypes.bfloat16
F32 = mybir.dt.float32
BF = mybir.dt.bfloat16

B, T, C = 4, 2048, 1024
NH, D = 16, 64          # global heads
HL = 8                  # local heads per core
G = 2                   # head groups (cores per batch)
FL = 3 * HL * D         # 1536 local qkv rows
CL = HL * D             # 512 local out channels
P = 128
TQ = 512                # query-block width
NTT = T // P            # 16 t-tiles
NPAIR = HL // 2         # 4 head pairs
NB = T // TQ            # 4 query blocks
KO = C // P             # 8 contraction slabs


def build_nc():
    nc = bacc.Bacc("TRN2", target_bir_lowering=False, debug=False, num_devices=8)

    xT = nc.declare_dram_parameter("xT", [C, T], BF, isOutput=False)
    wqkvT = nc.declare_dram_parameter("wqkvT", [C, FL], BF, isOutput=False)
    wprojT = nc.declare_dram_parameter("wprojT", [CL, C], BF, isOutput=False)
    cosd = nc.declare_dram_parameter("cosd", [P, T], BF, isOutput=False)
    sinu = nc.declare_dram_parameter("sinu", [P, T], BF, isOutput=False)
    y = nc.declare_dram_parameter("y", [T, C], F32, isOutput=True)

    Exp = mybir.ActivationFunctionType.Exp

    with TileContext(nc) as tc:
        with (
            tc.tile_pool(name="const", bufs=1) as const,
            tc.tile_pool(name="rope", bufs=4) as rope,
            tc.tile_pool(name="pexp", bufs=8) as pexp,
            tc.tile_pool(name="yout", bufs=3) as yout,
            tc.tile_pool(name="nrm", bufs=2) as nrm,
            tc.tile_pool(name="psscore", bufs=2, space="PSUM") as psscore,
            tc.tile_pool(name="psout", bufs=1, space="PSUM") as psout,
            tc.tile_pool(name="psmm", bufs=2, space="PSUM") as psmm,
        ):
            # ---- persistent SBUF tensors ----
            XT = const.tile([P, KO, T], BF, tag="XT")
            WQKV = const.tile([P, KO, FL], BF, tag="WQKV")
            WPROJ = const.tile([P, CL // P, C], BF, tag="WPROJ")
            COS = const.tile([P, T], BF, tag="COS")
            SINU = const.tile([P, T], BF, tag="SINU")
            V1 = const.tile([P, NTT, HL, D + 1], BF, tag="V1")
            QT = const.tile([P, NPAIR, T], BF, tag="QT")
            KT = const.tile([P, NPAIR, T], BF, tag="KT")
            ONORM = const.tile([P, NPAIR, T], BF, tag="ONORM")
            MTRI = const.tile([P, 2, TQ], BF, tag="MTRI")
            WARM = const.tile([P, TQ], BF, tag="WARM")

            # ---- input DMAs: first-needed data issues first (the 16 HW
            # queues run transfers concurrently, so late loads are only
            # delayed, never starved) ----
            xTr = xT.rearrange("(ko p) t -> p ko t", p=P)
            wqr = wqkvT.rearrange("(ko p) f -> p ko f", p=P)
            # Only t-block-0 data + the rope tables load up front; later
            # t-blocks and wproj are spliced into the pipeline as fillers
            # so the sync queue never head-of-line-blocks the rope shift
            # DMAs of the active block.
            # Weights load as f-slices in first-use order: the first qk
            # chain only needs its own 256KB column slice, so compute
            # starts ~3x earlier than with whole-slab loads.
            def emit_wq_load(fb, w):
                nc.sync.dma_start(WQKV[:, :, fb:fb + w], wqr[:, :, fb:fb + w])

            emit_wq_load(0, P)                # q, pair 0
            nc.sync.dma_start(COS[:, 0:TQ], cosd[:, 0:TQ])
            nc.sync.dma_start(SINU[:, 0:TQ], sinu[:, 0:TQ])
            for ko in range(KO):
                nc.sync.dma_start(XT[:, ko, 0:TQ], xTr[:, ko, 0:TQ])
            emit_wq_load(CL, P)               # k, pair 0
            emit_wq_load(2 * CL, CL)          # v, all heads
            nc.sync.dma_start(COS[:, TQ:T], cosd[:, TQ:T])
            nc.sync.dma_start(SINU[:, TQ:T], sinu[:, TQ:T])
            for p4 in range(1, NPAIR):
                emit_wq_load(p4 * P, P)
                emit_wq_load(CL + p4 * P, P)

            def emit_xt_load(tb):
                for ko in range(KO):
                    nc.sync.dma_start(
                        XT[:, ko, tb * TQ:(tb + 1) * TQ],
                        xTr[:, ko, tb * TQ:(tb + 1) * TQ])

            def emit_wproj_load():
                nc.sync.dma_start(
                    WPROJ[:], wprojT.rearrange("(ko p) o -> p ko o", p=P))

            # Warm-up: dense dependency-free matmuls on a memset tile keep
            # the PE's HAM clock gate at 8/8 while the input DMAs land
            # (results land in a scratch psum slot and are overwritten).
            nc.gpsimd.memset(WARM[:], 1.0)
            wps = psmm.tile([P, TQ], F32, tag="mm")
            for _ in range(8):
                nc.tensor.matmul(wps[:], lhsT=WARM[:, 0:P], rhs=WARM[:],
                                 start=True, stop=True)

            nc.gpsimd.memset(V1[:], 1.0)
            nc.gpsimd.memset(MTRI[:], 1.0)
            for h in range(2):  # universal triangular mask: keep f >= p
                nc.gpsimd.affine_select(
                    out=MTRI[:, h, :], in_=MTRI[:, h, :],
                    compare_op=mybir.AluOpType.is_ge, fill=0.0,
                    base=0, pattern=[[1, TQ]], channel_multiplier=-1)

            yr = y.rearrange("(n p) o -> p n o", p=P)

            # ---- emission helpers ----
            def emit_qk_chain(j, p4, qk):
                """Produce QT/KT pair p4 for t-block j, already transposed,
                with RoPE fused into the PSUM evacuation."""
                tsl = slice(j * TQ, (j + 1) * TQ)
                fb = qk * CL + p4 * P
                ps = psmm.tile([P, TQ], F32, tag="mm")
                for ko in range(KO):
                    nc.tensor.matmul(
                        ps[:], lhsT=WQKV[:, ko, fb:fb + P],
                        rhs=XT[:, ko, tsl],
                        start=(ko == 0), stop=(ko == KO - 1))
                a = rope.tile([P, TQ], BF, tag="ra")
                b = rope.tile([P, TQ], BF, tag="rb")
                bs = rope.tile([P, TQ], BF, tag="rs")
                nc.vector.tensor_mul(a[:], ps[:], COS[:, tsl])
                nc.vector.tensor_mul(b[:], ps[:], SINU[:, tsl])
                # rot_half = swap 32-partition blocks within each head
                for blk in range(4):
                    src = (blk ^ 1) * 32
                    nc.sync.dma_start(
                        bs[blk * 32:(blk + 1) * 32, :], b[src:src + 32, :])
                dst = QT if qk == 0 else KT
                nc.vector.tensor_add(dst[:, p4, tsl], a[:], bs[:])

            def emit_v_chain(i):
                ps = psmm.tile([P, TQ], F32, tag="mm")
                for ko in range(KO):
                    nc.tensor.matmul(
                        ps[:], lhsT=XT[:, ko, i * P:(i + 1) * P],
                        rhs=WQKV[:, ko, 2 * CL:3 * CL],
                        start=(ko == 0), stop=(ko == KO - 1))
                nc.vector.tensor_copy(
                    V1[:, i, :, 0:D], ps.rearrange("p (h d) -> p h d", h=HL))

            def emit_att_m(j, p4, m, ntk, outAB):
                # Diagonal-straddling tiles (ml >= 0) only contribute to
                # queries tq >= 128*ml within the block: shrink the score
                # matmuls, exp, mask, and attn@v to that span.  Since the
                # span offset equals 128*ml, the causal mask reduces to a
                # universal triangle (keep f >= p) for every such tile.
                ml = m - (TQ // P) * j
                qoff = max(ml, 0) * P
                nq = TQ - qoff
                qsl = slice(j * TQ + qoff, (j + 1) * TQ)
                ksl = slice(m * P, (m + 1) * P)
                st = psscore.tile([P, 2 * TQ], F32, tag="st")
                st2 = st.rearrange("p (h q) -> p h q", h=2)
                nc.tensor.matmul(
                    st2[:, 0, qoff:TQ], lhsT=KT[0:D, p4, ksl],
                    rhs=QT[0:D, p4, qsl],
                    start=True, stop=True)
                nc.tensor.matmul(
                    st2[:, 1, qoff:TQ], lhsT=KT[D:P, p4, ksl],
                    rhs=QT[D:P, p4, qsl],
                    start=True, stop=True, tile_position=(D, 0))
                pt = pexp.tile([P, 2 * TQ], BF, tag="pt")
                pt2 = pt.rearrange("p (h q) -> p h q", h=2)
                nc.scalar.activation(
                    pt2[:, :, qoff:TQ], st2[:, :, qoff:TQ], Exp, scale=0.125)
                if ml >= 0:  # zero tk > tq on the diagonal tile
                    nc.vector.tensor_mul(
                        pt2[:, :, qoff:TQ], pt2[:, :, qoff:TQ],
                        MTRI[:, :, 0:nq])
                nc.tensor.matmul(
                    outAB[0:D + 1, qoff:TQ], lhsT=V1[:, m, 2 * p4, :],
                    rhs=pt2[:, 0, qoff:TQ],
                    start=(m == 0), stop=(m == ntk - 1))
                nc.tensor.matmul(
                    outAB[0:D + 1, TQ + qoff:2 * TQ],
                    lhsT=V1[:, m, 2 * p4 + 1, :],
                    rhs=pt2[:, 1, qoff:TQ],
                    start=(m == 0), stop=(m == ntk - 1))

            def emit_norm(j, p4, outAB):
                qsl = slice(j * TQ, (j + 1) * TQ)
                l2 = nrm.tile([1, 2 * TQ], F32, tag="l2")
                nc.vector.tensor_copy(l2[:], outAB[D:D + 1, :])
                r2 = nrm.tile([1, 2 * TQ], F32, tag="r2")
                nc.vector.reciprocal_approx_fast(out=r2[:], in_=l2[:])
                for w in range(2):
                    r64 = nrm.tile([D, TQ], F32, tag="r64")
                    nc.gpsimd.partition_broadcast(
                        r64[:], r2[:, w * TQ:(w + 1) * TQ])
                    nc.vector.tensor_mul(
                        ONORM[w * D:(w + 1) * D, p4, qsl],
                        outAB[0:D, w * TQ:(w + 1) * TQ], r64[:])

            def emit_proj(i, n2):
                ps = psmm.tile([P, TQ], F32, tag="mm")
                for kc in range(NPAIR):
                    nc.tensor.matmul(
                        ps[:], lhsT=ONORM[:, kc, i * P:(i + 1) * P],
                        rhs=WPROJ[:, kc, n2 * TQ:(n2 + 1) * TQ],
                        start=(kc == 0), stop=(kc == NPAIR - 1))
                ysb = yout.tile([P, TQ], F32, tag="ysb")
                nc.vector.tensor_copy(ysb[:], ps[:])
                # output stores ride the software DGE: latency-tolerant,
                # keeps the sync queue free for rope shifts
                nc.gpsimd.dma_start(yr[:, i, n2 * TQ:(n2 + 1) * TQ], ysb[:])

            def q_fillers(j):
                return [lambda j=j, p4=p4: emit_qk_chain(j, p4, 0)
                        for p4 in range(NPAIR)]

            def qk_fillers(j):
                # pair-interleaved so pair 0 of the next block completes
                # first (its scores unlock that block's exp stream)
                f = []
                for p4 in range(NPAIR):
                    f.append(lambda j=j, p4=p4: emit_qk_chain(j, p4, 0))
                    f.append(lambda j=j, p4=p4: emit_qk_chain(j, p4, 1))
                return f

            def kv_fillers(j):
                f = [lambda j=j, p4=p4: emit_qk_chain(j, p4, 1)
                     for p4 in range(NPAIR)]
                f += [lambda i=j * (TQ // P) + s: emit_v_chain(i)
                      for s in range(TQ // P)]
                return f

            def v_fillers(j):
                return [lambda i=j * (TQ // P) + s: emit_v_chain(i)
                        for s in range(TQ // P)]

            def proj_block_fillers(j):
                return [lambda i=i, n2=n2: emit_proj(i, n2)
                        for i in range(j * (TQ // P), (j + 1) * (TQ // P))
                        for n2 in range(C // TQ)]

            # ---- software-pipelined main loop ----
            # Fillers (independent PE chains) are spliced between the
            # ScalarE-bound attention iterations of each block so the PE
            # never starves.  Later blocks are ACT-bound, so the late
            # qkv/proj chains are pulled as early as dependencies allow.

            # attention(0) starts as soon as pair 0 + v of block 0 exist;
            # the q/k chains of later pairs are spliced into earlier groups
            emit_qk_chain(0, 0, 0)
            emit_qk_chain(0, 0, 1)
            for s in range(TQ // P):
                emit_v_chain(s)
            filler_map = {
                0: [lambda: emit_xt_load(1)] + qk_fillers(1) + v_fillers(1)
                   + [lambda: emit_xt_load(2)],
                1: [lambda: emit_xt_load(3), emit_wproj_load]
                   + qk_fillers(2) + v_fillers(2) + q_fillers(3),
                2: kv_fillers(3) + proj_block_fillers(0),
                3: proj_block_fillers(1) + proj_block_fillers(2),
            }
            carry = []
            for j in range(NB):
                fillers = carry + filler_map[j]
                ntk = (TQ // P) * (j + 1)
                # front-load fillers over ~70% of the block so qkv(j+1)
                # finishes before attention(j) drains ScalarE's queue
                natt = (NPAIR * ntk * 7) // 10
                fi = 0
                k = 0
                for p4 in range(NPAIR):
                    outAB = psout.tile([D + 1, 2 * TQ], F32, tag="out")
                    for m in range(ntk):
                        emit_att_m(j, p4, m, ntk, outAB)
                        if j == 0 and p4 < NPAIR - 1 and m < 2:
                            emit_qk_chain(0, p4 + 1, m)
                        k += 1
                        while fi < len(fillers) and fi * natt < k * len(fillers):
                            fillers[fi]()
                            fi += 1
                    emit_norm(j, p4, outAB)
                # leftovers ride along with the next block's fillers so the
                # next block's score/exp stream is not delayed behind them
                carry = fillers[fi:]
            for fn in carry:
                fn()
            for fn in proj_block_fillers(NB - 1):
                fn()

    nc.compile()
    return nc


def prep_inputs(x, w_qkv, w_proj):
    """Build the 8 per-core input maps from the full-problem inputs."""
    x = np.asarray(x, dtype=np.float32)
    w_qkv = np.asarray(w_qkv, dtype=np.float32)
    w_proj = np.asarray(w_proj, dtype=np.float32)

    inv_freq = 1.0 / (10000.0 ** (np.arange(0, D, 2, dtype=np.float32) / D))
    tt = np.arange(T, dtype=np.float32)
    freqs = np.outer(tt, inv_freq).astype(np.float32)  # [T, 32]
    cos_t = np.cos(freqs).astype(np.float32)           # [T, 32]
    sin_t = np.sin(freqs).astype(np.float32)
    # [d, t] tables for the transposed layout, stacked for a head pair.
    # cosd[p, t] = cos(f[t, p%32]); sinu carries rot_half's sign:
    # +sin for p%64 in [0,32) (source for upper target), -sin for [32,64).
    cos64 = np.concatenate([cos_t.T, cos_t.T], axis=0)   # [64, T]
    sin64 = np.concatenate([sin_t.T, -sin_t.T], axis=0)  # [64, T]
    cosd = np.ascontiguousarray(
        np.concatenate([cos64, cos64], axis=0)).astype(BF16)  # [128, T]
    sinu = np.ascontiguousarray(
        np.concatenate([sin64, sin64], axis=0)).astype(BF16)

    in_maps = []
    for core in range(8):
        b, g = divmod(core, G)
        sl = slice(g * CL, (g + 1) * CL)
        w_local = np.concatenate(
            [w_qkv[sl], w_qkv[C:][sl], w_qkv[2 * C:][sl]], axis=0)  # [1536, C]
        in_maps.append({
            "xT": np.ascontiguousarray(x[b].T).astype(BF16),
            "wqkvT": np.ascontiguousarray(w_local.T).astype(BF16),
            "wprojT": np.ascontiguousarray(w_proj[:, sl].T).astype(BF16),
            "cosd": cosd,
            "sinu": sinu,
        })
    return in_maps


_NC_LOCK = threading.Lock()
_NC = None


def get_nc():
    global _NC
    with _NC_LOCK:
        if _NC is None:
            _NC = build_nc()
    return _NC


def run(nc, in_maps, **kw):
    res = run_bass_kernel_spmd(nc, in_maps, list(range(8)), **kw)
    parts = [res.results[c]["y"] for c in range(8)]
    out = np.stack([parts[2 * b] + parts[2 * b + 1] for b in range(B)])
    return out.astype(np.float32), res


def kernel(x, w_qkv, w_proj):
    out, _ = run(get_nc(), prep_inputs(x, w_qkv, w_proj))
    return out
